# revision 6
# baseline (speedup 1.0000x reference)
"""Trainium2 Bass kernel for single-token decode attention (sparse_attention).

Tensor-parallel over heads: 32 heads sharded 4-per-core across 8 NeuronCores.
Each core computes QKV projection + RoPE + cache-attention + output projection
for its 4 heads; the host sums the 8 partial output projections (the
"all-reduce" of the final 'anh,nhd->ad' einsum) and scatters the new K/V row
into host-side copies of the caches.

Self-contained: hardcodes all shapes from the problem spec.
"""

import numpy as np

import concourse.bass as bass
import concourse.mybir as mybir
import concourse.tile as tile
from concourse import bacc
from concourse.bass_utils import run_bass_kernel_spmd

F32 = mybir.dt.float32

NCORES = 8
B, T, D, N, H, S = 1, 1, 4096, 32, 128, 4096
NH = N // NCORES            # heads per core = 4
M = 3 * NH                  # qkv matrices per core = 12
DT = D // 128               # d tiles = 32
ST = S // 128               # s tiles = 32
WCH = 2                     # d-tiles per w_qkv DMA chunk

MAX_WAVELENGTH = 10000.0

# Cache of built+compiled Bass programs keyed by baked scalar (time_step).
_NC_CACHE: dict[int, object] = {}

# Set by test harness to request a profiled run; exec time lands in LAST_EXEC_NS.
TRACE = False
LAST_EXEC_NS = None


def _build_nc(ts_i: int):
    """Build the per-core Bass program. ts_i (= time_step) is baked in: it only
    selects which PSUM logit element / V-tile row gets overwritten with the
    newly-projected K/V token."""
    p0 = ts_i % 128            # partition (s within tile)
    c0 = ts_i // 128           # s-tile index (0..31)
    b0 = c0 // 4               # which vt DMA chunk holds the row
    a0 = c0 % 4                # sub-chunk within the vt DMA

    nc = bacc.Bacc("TRN2", target_bir_lowering=False, debug=False)

    # ---- DRAM I/O (per-core shard layouts, prepared host-side) ----
    # xt[p, t]      = x[t*128 + p]
    xt_d = nc.dram_tensor("xt", [128, DT], F32, kind="ExternalInput")
    # w[t, p, m*128+h] = w_qkv[qkv, head, t*128+p, h]   (m = qkv*NH + head)
    w_d = nc.dram_tensor("w", [DT, 128, M * 128], F32, kind="ExternalInput")
    # kt[n, h, s]   = k_cache[s, head, h]
    kt_d = nc.dram_tensor("kt", [NH, H, S], F32, kind="ExternalInput")
    # vt[t, p, n*128+h] = v_cache[t*128+p, head, h]
    vt_d = nc.dram_tensor("vt", [ST, 128, NH * H], F32, kind="ExternalInput")
    # wo[n, h, d]   = w_out[head, h, d]
    wo_d = nc.dram_tensor("wo", [NH, H, D], F32, kind="ExternalInput")
    # maskt[p, t]   = attn_mask[t*128 + p]
    mask_d = nc.dram_tensor("maskt", [128, ST], F32, kind="ExternalInput")
    # RoPE per-partition columns: cos/sin indexed by h%64; q versions carry H^-0.5
    cq_d = nc.dram_tensor("cosq", [128, 1], F32, kind="ExternalInput")
    sq_d = nc.dram_tensor("sinq", [128, 1], F32, kind="ExternalInput")
    ck_d = nc.dram_tensor("cosk", [128, 1], F32, kind="ExternalInput")
    sk_d = nc.dram_tensor("sink", [128, 1], F32, kind="ExternalInput")
    # RoPE half-swap permutation: out[m] = sum_k P[k, m] * in[k]
    pm_d = nc.dram_tensor("pmat", [128, 128], F32, kind="ExternalInput")

    # outp[j, t]  = partial attn_output[t*128 + j]
    outp_d = nc.dram_tensor("outp", [128, DT], F32, kind="ExternalOutput")
    # knew[h, n] = roped k row for head n ; vnew[h, n] = v row
    knew_d = nc.dram_tensor("knew", [128, NH], F32, kind="ExternalOutput")
    vnew_d = nc.dram_tensor("vnew", [128, NH], F32, kind="ExternalOutput")

    with tile.TileContext(nc) as tc:
        with (
            tc.tile_pool(name="consts", bufs=1) as consts,
            tc.tile_pool(name="wp", bufs=3) as wp,
            tc.tile_pool(name="ktp", bufs=2) as ktp,
            tc.tile_pool(name="vtp", bufs=2) as vtp,
            tc.tile_pool(name="wop", bufs=2) as wop,
            tc.tile_pool(name="work", bufs=1) as work,
            tc.tile_pool(name="psum", bufs=1, space="PSUM") as psum,
        ):
            # ---- constants ----
            xt_s = consts.tile([128, DT], F32, tag="xt_s")
            nc.sync.dma_start(out=xt_s, in_=xt_d[:])
            pm_s = consts.tile([128, 128], F32, tag="pm_s")
            nc.sync.dma_start(out=pm_s, in_=pm_d[:])
            cq_s = consts.tile([128, 1], F32, tag="cq_s")
            nc.sync.dma_start(out=cq_s, in_=cq_d[:])
            sq_s = consts.tile([128, 1], F32, tag="sq_s")
            nc.sync.dma_start(out=sq_s, in_=sq_d[:])
            ck_s = consts.tile([128, 1], F32, tag="ck_s")
            nc.sync.dma_start(out=ck_s, in_=ck_d[:])
            sk_s = consts.tile([128, 1], F32, tag="sk_s")
            nc.sync.dma_start(out=sk_s, in_=sk_d[:])
            mask_s = consts.tile([128, ST], F32, tag="mask_s")
            nc.sync.dma_start(out=mask_s, in_=mask_d[:])
            ones_col = consts.tile([128, 1], F32, tag="ones_col")
            nc.vector.memset(ones_col, 1.0)
            ones_row = consts.tile([1, 128], F32, tag="ones_row")
            nc.vector.memset(ones_row, 1.0)

            # ---- QKV projection: psum_qkv[h, m] = sum_d w[d, m, h] * x[d] ----
            ps_qkv = psum.tile([128, M], F32, tag="ps_qkv")
            for c in range(DT // WCH):
                wt = wp.tile([128, WCH, M * 128], F32, tag="wt")
                nc.sync.dma_start(
                    out=wt, in_=w_d[c * WCH:(c + 1) * WCH].rearrange("a p f -> p a f")
                )
                for a in range(WCH):
                    d = c * WCH + a
                    for m in range(M):
                        nc.tensor.matmul(
                            out=ps_qkv[:, m:m + 1],
                            lhsT=wt[:, a, m * 128:(m + 1) * 128],
                            rhs=xt_s[:, d:d + 1],
                            start=(d == 0 and m == 0),
                            stop=(d == DT - 1 and m == M - 1),
                        )

            # ---- RoPE ----
            qkv_s = work.tile([128, M], F32, tag="qkv_s")
            nc.vector.tensor_copy(out=qkv_s, in_=ps_qkv)
            # rotated halves of q and k (cols 0..2*NH): rot[h] = -in[h+64] (h<64),
            # rot[h] = in[h-64] (h>=64); sign baked into pmat.
            ps_rot = psum.tile([128, 2 * NH], F32, tag="ps_rot")
            nc.tensor.matmul(
                out=ps_rot, lhsT=pm_s, rhs=qkv_s[:, 0:2 * NH], start=True, stop=True
            )
            qr_s = work.tile([128, NH], F32, tag="qr_s")
            kr_s = work.tile([128, NH], F32, tag="kr_s")
            tmp_s = work.tile([128, 2 * NH], F32, tag="tmp_s")
            # q_roped = q*cos*H^-0.5 + rot(q)*sin*H^-0.5 ; k_roped = k*cos + rot(k)*sin
            nc.vector.tensor_scalar_mul(out=qr_s, in0=qkv_s[:, 0:NH], scalar1=cq_s)
            nc.vector.tensor_scalar_mul(
                out=tmp_s[:, 0:NH], in0=ps_rot[:, 0:NH], scalar1=sq_s
            )
            nc.vector.tensor_add(out=qr_s, in0=qr_s, in1=tmp_s[:, 0:NH])
            nc.vector.tensor_scalar_mul(out=kr_s, in0=qkv_s[:, NH:2 * NH], scalar1=ck_s)
            nc.vector.tensor_scalar_mul(
                out=tmp_s[:, NH:2 * NH], in0=ps_rot[:, NH:2 * NH], scalar1=sk_s
            )
            nc.vector.tensor_add(out=kr_s, in0=kr_s, in1=tmp_s[:, NH:2 * NH])
            # new-token outputs
            nc.sync.dma_start(out=knew_d[:], in_=kr_s)
            nc.sync.dma_start(out=vnew_d[:], in_=qkv_s[:, 2 * NH:3 * NH])

            # logit of the new token against itself: qk_new[n] = sum_h q[h,n]*k[h,n]
            prod_s = work.tile([128, NH], F32, tag="prod_s")
            nc.vector.tensor_mul(out=prod_s, in0=qr_s, in1=kr_s)
            ps_qkn = psum.tile([1, NH], F32, tag="ps_qkn")
            nc.tensor.matmul(out=ps_qkn, lhsT=ones_col, rhs=prod_s, start=True, stop=True)
            qkn_s = work.tile([1, NH], F32, tag="qkn_s")
            nc.vector.tensor_copy(out=qkn_s, in_=ps_qkn)

            # ---- logits: ps_lg[p, n, t] = sum_h kt[n, h, t*128+p] * q[h, n] ----
            ps_lg = psum.tile([128, NH, ST], F32, tag="ps_lg")
            for bch in range(8):
                ktt = ktp.tile([128, NH, 512], F32, tag="ktt")
                nc.sync.dma_start(
                    out=ktt,
                    in_=kt_d[:, :, bch * 512:(bch + 1) * 512].rearrange("n h s -> h n s"),
                )
                for n in range(NH):
                    for t in range(4):
                        sti = bch * 4 + t
                        nc.tensor.matmul(
                            out=ps_lg[:, n, sti:sti + 1],
                            lhsT=ktt[:, n, t * 128:(t + 1) * 128],
                            rhs=qr_s[:, n:n + 1],
                            start=True,
                            stop=True,
                        )

            # mask-add into SBUF, overwrite the time_step element, exponentiate
            lg_s = work.tile([128, NH, ST], F32, tag="lg_s")
            for n in range(NH):
                nc.vector.tensor_add(out=lg_s[:, n, :], in0=ps_lg[:, n, :], in1=mask_s)
            nc.gpsimd.dma_start(out=lg_s[p0:p0 + 1, :, c0], in_=qkn_s)
            e_s = work.tile([128, NH, ST], F32, tag="e_s")
            dsum = work.tile([128, NH], F32, tag="dsum")
            for n in range(NH):
                nc.scalar.activation(
                    out=e_s[:, n, :],
                    in_=lg_s[:, n, :],
                    func=mybir.ActivationFunctionType.Exp,
                    accum_out=dsum[:, n:n + 1],
                )

            # ---- encoded: ps_enc[h, n] = sum_s v[s, n, h] * e[s, n] ----
            ps_enc = psum.tile([128, NH], F32, tag="ps_enc")
            for bch in range(8):
                vtt = vtp.tile([128, 4, NH * H], F32, tag="vtt")
                nc.sync.dma_start(
                    out=vtt,
                    in_=vt_d[bch * 4:(bch + 1) * 4].rearrange("a p f -> p a f"),
                )
                if bch == b0:
                    # overwrite cache row time_step with the new v (h lives on
                    # partitions in qkv_s -> gather to one partition per head)
                    for n in range(NH):
                        nc.gpsimd.dma_start(
                            out=vtt[p0:p0 + 1, a0, n * H:(n + 1) * H],
                            in_=qkv_s[:, 2 * NH + n:2 * NH + n + 1],
                        )
                for a in range(4):
                    sti = bch * 4 + a
                    for n in range(NH):
                        nc.tensor.matmul(
                            out=ps_enc[:, n:n + 1],
                            lhsT=vtt[:, a, n * H:(n + 1) * H],
                            rhs=e_s[:, n, sti:sti + 1],
                            start=(sti == 0 and n == 0),
                            stop=(sti == ST - 1 and n == NH - 1),
                        )

            # ---- softmax denominator ----
            ps_d = psum.tile([1, NH], F32, tag="ps_d")
            nc.tensor.matmul(out=ps_d, lhsT=ones_col, rhs=dsum, start=True, stop=True)
            rcp_s = work.tile([1, NH], F32, tag="rcp_s")
            nc.vector.reciprocal(out=rcp_s, in_=ps_d)
            ps_b = psum.tile([128, NH], F32, tag="ps_b")
            nc.tensor.matmul(out=ps_b, lhsT=ones_row, rhs=rcp_s, start=True, stop=True)
            rcpb_s = work.tile([128, NH], F32, tag="rcpb_s")
            nc.vector.tensor_copy(out=rcpb_s, in_=ps_b)
            enc_s = work.tile([128, NH], F32, tag="enc_s")
            nc.vector.tensor_mul(out=enc_s, in0=ps_enc, in1=rcpb_s)

            # ---- output projection: ps_o[j, t] = sum_{n,h} wo[n, h, t*128+j]*enc[h, n]
            ps_o = psum.tile([128, DT], F32, tag="ps_o")
            for n in range(NH):
                wot = wop.tile([128, D], F32, tag="wot")
                nc.sync.dma_start(out=wot, in_=wo_d[n])
                for t in range(DT):
                    nc.tensor.matmul(
                        out=ps_o[:, t:t + 1],
                        lhsT=wot[:, t * 128:(t + 1) * 128],
                        rhs=enc_s[:, n:n + 1],
                        start=(n == 0 and t == 0),
                        stop=(n == NH - 1 and t == DT - 1),
                    )
            out_s = work.tile([128, DT], F32, tag="out_s")
            nc.vector.tensor_copy(out=out_s, in_=ps_o)
            nc.sync.dma_start(out=outp_d[:], in_=out_s)

    nc.compile()
    return nc


def _get_nc(ts_i: int):
    if ts_i not in _NC_CACHE:
        _NC_CACHE[ts_i] = _build_nc(ts_i)
    return _NC_CACHE[ts_i]


def _host_shards(x, w_qkv, w_out, k_cache, v_cache, attn_mask, sp_i):
    """Build the 8 per-core input maps (all host-side numpy reshapes)."""
    f4 = np.float32
    xt = np.ascontiguousarray(x.reshape(D).reshape(DT, 128).T, dtype=f4)
    maskt = np.ascontiguousarray(
        attn_mask.reshape(S).reshape(ST, 128).T, dtype=f4
    )

    # RoPE constants from segment_pos
    fraction = 2.0 * np.arange(H // 2, dtype=np.float64) / H
    timescale = MAX_WAVELENGTH ** fraction
    sinusoid = float(sp_i) / timescale
    sin64 = np.sin(sinusoid).astype(f4)
    cos64 = np.cos(sinusoid).astype(f4)
    cos_col = np.concatenate([cos64, cos64]).reshape(128, 1)
    sin_col = np.concatenate([sin64, sin64]).reshape(128, 1)
    hs = np.float32(H ** -0.5)
    cq = np.ascontiguousarray(cos_col * hs)
    sq = np.ascontiguousarray(sin_col * hs)
    ck = np.ascontiguousarray(cos_col)
    sk = np.ascontiguousarray(sin_col)

    pm = np.zeros((128, 128), dtype=f4)
    for mi in range(64):
        pm[mi + 64, mi] = -1.0   # out[m] = -in[m+64]
        pm[mi, mi + 64] = 1.0    # out[m+64] = in[m]

    # [D, 3, N, H] view of w_qkv for cheap per-core slicing
    w_all = np.ascontiguousarray(w_qkv.transpose(2, 0, 1, 3))
    kt_all = np.ascontiguousarray(k_cache.transpose(1, 2, 0))  # [N, H, S]

    in_maps = []
    for core in range(NCORES):
        h0 = core * NH
        hsl = slice(h0, h0 + NH)
        w_c = np.ascontiguousarray(w_all[:, :, hsl, :]).reshape(DT, 128, M * 128)
        kt_c = np.ascontiguousarray(kt_all[hsl])
        vt_c = np.ascontiguousarray(v_cache[:, hsl, :]).reshape(ST, 128, NH * H)
        wo_c = np.ascontiguousarray(w_out[hsl])
        in_maps.append(
            {
                "xt": xt, "w": w_c, "kt": kt_c, "vt": vt_c, "wo": wo_c,
                "maskt": maskt, "cosq": cq, "sinq": sq, "cosk": ck, "sink": sk,
                "pmat": pm,
            }
        )
    return in_maps


def kernel(x, w_qkv, w_out, k_cache, v_cache, attn_mask, segment_pos, time_step):
    global LAST_EXEC_NS
    x = np.asarray(x, dtype=np.float32)
    w_qkv = np.asarray(w_qkv, dtype=np.float32)
    w_out = np.asarray(w_out, dtype=np.float32)
    k_cache = np.asarray(k_cache, dtype=np.float32)
    v_cache = np.asarray(v_cache, dtype=np.float32)
    attn_mask = np.asarray(attn_mask, dtype=np.float32)
    sp_i = int(np.asarray(segment_pos))
    ts_i = int(np.asarray(time_step))

    nc = _get_nc(ts_i)
    in_maps = _host_shards(x, w_qkv, w_out, k_cache, v_cache, attn_mask, sp_i)

    res = run_bass_kernel_spmd(nc, in_maps, list(range(NCORES)))

    # ---- gather / unshard ----
    attn = np.zeros((D,), dtype=np.float64)
    k_out = k_cache.copy()
    v_out = v_cache.copy()
    for core in range(NCORES):
        r = res.results[core]
        attn += r["outp"].T.reshape(D).astype(np.float64)
        k_out[ts_i, core * NH:(core + 1) * NH, :] = r["knew"].T
        v_out[ts_i, core * NH:(core + 1) * NH, :] = r["vnew"].T
    attn_out = attn.astype(np.float32).reshape(1, D)
    return k_out, v_out, attn_out
